# revision 6
# baseline (speedup 1.0000x reference)
"""DecorrLoss distributed Trainium2 kernel.

Reference math (x: (2, 4096, 128) f32, kappa: scalar):
    xf = x.reshape(-1, d)          # n = 8192 samples, d = 128 features
    x2 = xf * xf
    s2_i = sum_j x2[i, j]          # per-sample row sums
    s4_i = sum_j x2[i, j]^2
    corr_loss = (sum_i s2_i^2 - sum_i s4_i) / (n d^2)
    whit_loss = (S4 - 2 S2 + n d) / (n d^2)   with S2 = sum x2, S4 = sum x2^2
    G = xf.T @ xf / n
    grad = (1-kappa) * (G with zeroed diag) + kappa * diag(diag(G) - 1)

Strategy: data-parallel over 8 NeuronCores, 1024 samples each. Each core
computes a partial X^T X (PSUM-accumulated fp32 matmuls), per-partition
partial sums of s2^2 (scalar engine: Square activation with accumulate)
and s4 (vector engine: fused multiply-reduce). The tiny (128, 130)
per-core partials are summed on host: diag of X^T X supplies the column
sums of x^2, so S2 = trace and mean(x^2) need no extra device work.
"""

from contextlib import ExitStack

import numpy as np

import concourse.bacc as bacc
import concourse.mybir as mybir
import concourse.tile as tile
from concourse.bass_utils import run_bass_kernel_spmd

N_CORES = 8
N = 8192          # total flattened samples
D = 128           # feature dim
NS = N // N_CORES  # samples per core
T = NS // 128      # 128-sample tiles per core

_nc = None


def _build():
    """Build the per-core Bass graph (identical on all 8 cores)."""
    nc = bacc.Bacc(None, target_bir_lowering=False)
    x_ext = nc.dram_tensor("x", [T, 128, D], mybir.dt.float32, kind="ExternalInput")
    out_ext = nc.dram_tensor(
        "out", [128, D + 2], mybir.dt.float32, kind="ExternalOutput"
    )

    with tile.TileContext(nc) as tc, ExitStack() as ctx:
        xp = ctx.enter_context(tc.tile_pool(name="xp", bufs=4))
        jp = ctx.enter_context(tc.tile_pool(name="jp", bufs=2))
        sp = ctx.enter_context(tc.tile_pool(name="sp", bufs=1))
        pp = ctx.enter_context(tc.tile_pool(name="pp", bufs=1, space="PSUM"))

        s2all = sp.tile([128, T], mybir.dt.float32)   # per-tile row sums of x^2
        s4all = sp.tile([128, T], mybir.dt.float32)   # per-tile row sums of x^4
        gps = pp.tile([128, D], mybir.dt.float32)     # partial X^T X accumulator

        for t in range(T):
            xt = xp.tile([128, D], mybir.dt.float32, tag="xt")
            nc.sync.dma_start(out=xt[:], in_=x_ext[t, :, :])
            nc.tensor.matmul(
                gps[:], xt[:], xt[:], start=(t == 0), stop=(t == T - 1)
            )
            x2 = jp.tile([128, D], mybir.dt.float32, tag="x2")
            nc.scalar.activation(
                out=x2[:],
                in_=xt[:],
                func=mybir.ActivationFunctionType.Square,
                accum_out=s2all[:, t : t + 1],
            )
            x4 = jp.tile([128, D], mybir.dt.float32, tag="x4")
            nc.vector.scalar_tensor_tensor(
                out=x4[:],
                in0=x2[:],
                scalar=1.0,
                in1=x2[:],
                op0=mybir.AluOpType.mult,
                op1=mybir.AluOpType.mult,
                accum_out=s4all[:, t : t + 1],
            )

        outsb = sp.tile([128, D + 2], mybir.dt.float32)
        nc.vector.tensor_copy(out=outsb[:, 0:D], in_=gps[:])
        j2 = sp.tile([128, T], mybir.dt.float32)
        nc.vector.scalar_tensor_tensor(
            out=j2[:],
            in0=s2all[:],
            scalar=1.0,
            in1=s2all[:],
            op0=mybir.AluOpType.mult,
            op1=mybir.AluOpType.mult,
            accum_out=outsb[:, D : D + 1],
        )
        nc.vector.reduce_sum(
            out=outsb[:, D + 1 : D + 2], in_=s4all[:], axis=mybir.AxisListType.X
        )
        nc.sync.dma_start(out=out_ext[:], in_=outsb[:])
    nc.compile()
    return nc


def _get_nc():
    global _nc
    if _nc is None:
        _nc = _build()
    return _nc


def run_device(x, trace=False):
    """Run the SPMD kernel on 8 cores; return (per-core partials, results obj)."""
    xf = np.ascontiguousarray(np.asarray(x, dtype=np.float32).reshape(N, D))
    in_maps = [
        {"x": xf[i * NS : (i + 1) * NS].reshape(T, 128, D)} for i in range(N_CORES)
    ]
    res = run_bass_kernel_spmd(
        _get_nc(), in_maps, core_ids=list(range(N_CORES)), trace=trace
    )
    parts = np.stack([r["out"] for r in res.results])  # (8, 128, D+2)
    return parts, res


def finalize(parts, kappa):
    """Host-side reduction of per-core partials to (grad, corr, whit)."""
    kappa = float(np.asarray(kappa))
    M = parts.astype(np.float64).sum(axis=0)  # (128, D+2)
    Gs = M[:, :D]            # sum_i x_i x_i^T over all n samples
    q = M[:, D].sum()        # sum_i s2_i^2
    S4 = M[:, D + 1].sum()   # sum of x^4 over everything
    n = float(N)
    d = float(D)
    S2 = np.trace(Gs)        # sum of x^2 over everything
    corr = (q - S4) / (n * d * d)
    whit = (S4 - 2.0 * S2 + n * d) / (n * d * d)
    G = Gs / n
    eye = np.eye(D)
    # diag(G) = mean(x^2, axis=0) because the diagonal of X^T X is the column
    # sum of x^2
    grad = (1.0 - kappa) * (G * (1.0 - eye)) + kappa * np.diag(np.diag(G) - 1.0)
    return (
        grad.astype(np.float32),
        np.float32(corr),
        np.float32(whit),
    )


def kernel(x, kappa):
    parts, _ = run_device(x, trace=False)
    return finalize(parts, kappa)
